# revision 29
# baseline (speedup 1.0000x reference)
"""TRN2 Bass kernel for nn_MultiHeadAttention_79714593014244.

Reference math (per token n, NOT sequence attention):
    Q = x @ W_q, K = x @ W_k, V = x @ W_v          (x: [N, 4096])
    S[n] = Q[n] @ K[n].T        over heads          ([32, 32] per token)
    A[n] = softmax(S[n], axis=-1)
    y[n] = A[n] @ V[n]
    out = y.reshape(N, 4096) @ W_o
Sharding: pure data-parallel over tokens across 8 cores.

Per-core plan (projections fp32r, W_o pass bf16):
  Phase A: xT [128, 32 kc, 1024 tok] DMA'd directly (x host-transposed).
           For W in (W_q, W_k, W_v): stream W column-chunks, compute
           [feat, tok] projections (PSUM-accumulated over 32 K-chunks),
           spill to DRAM. Q/K spill as [head, d, tok]; V spills
           token-interleaved as [4*32 (tmod, g), d, tok/4] so that a
           4-token V block later loads as 128 partitions directly.
  Phase B (per 512-token half): attention, 4 tokens per PE instruction.
           For each group of 4 tokens:
             - ONE matmul S[(t,h) 128, (t',g) 128] (contract d=128) via
               strided APs picking 4 tokens from the [d, h, tok] slices.
               Only diagonal 32x32 blocks are wanted; off-diag is
               cross-token garbage.
             - DVE tensor_tensor_reduce: sm = -(S + M) with M the
               -1e30 off-block-diagonal mask, accum min -> negmax
               (= -(per-row max over the valid block)).
             - ONE scalar Exp [128,128]: e = exp(-sm + negmax), i.e.
               exp(S+M - max); off-diag -> exp(-1e30)=0; accum -> denom.
             - DVE reciprocal; Pool tensor_scalar_mul: a = e * recip
               (fp32r, block-diagonal).
             - ONE PE transpose a -> aT [(t',g), (t,h)].
             - ONE matmul yT[d 128, (t,h) 128] = V4.T @ aT (contract
               (t',g)=128) with V4 from the interleaved spill.
             - Pool copy into yt [128 d, 32 h, 512 tok] (bf16).
           PE work is software-pipelined: transpose/y-matmul for group
           g are emitted LAG groups behind the S-matmul so the PE queue
           never stalls on the softmax round-trip.
  Phase C (per half): stream bf16 W_o column-chunks, out = y @ W_o.
"""

import os

import ml_dtypes
import numpy as np

import concourse.bass as bass
import concourse.tile as tile
from concourse import bacc, mybir
from concourse.bass_utils import run_bass_kernel_spmd

N_CORES = 8
N_TOKENS = 8192
DIM = 4096
H = 32  # heads
D = 128  # head dim
KC = DIM // 128  # contraction chunks (32)
TOK = N_TOKENS // N_CORES  # tokens per core (1024)
HALF = 512  # tokens per B+C fusion block
MACRO = 128  # tokens per attention slice load
NG = MACRO // 4  # 4-token groups per macro (32)
GH = HALF // 4  # groups per half (128)
HB = 16  # groups per Q-repack block
LAG = 4  # PE pipeline lag (groups) between S-matmul and transpose
F32R = mybir.dt.float32r
F32 = mybir.dt.float32
BF16 = mybir.dt.bfloat16

_NC_CACHE = {}


def _build_nc():
    nc = bacc.Bacc(None, target_bir_lowering=False)

    xt_d = nc.dram_tensor("xt", [DIM, TOK], F32R, kind="ExternalInput")
    wq_d = nc.dram_tensor("wq", [DIM, DIM], F32R, kind="ExternalInput")
    wk_d = nc.dram_tensor("wk", [DIM, DIM], F32R, kind="ExternalInput")
    wv_d = nc.dram_tensor("wv", [DIM, DIM], F32R, kind="ExternalInput")
    wo_d = nc.dram_tensor("wo_bf16", [DIM, DIM], BF16, kind="ExternalInput")
    id_d = nc.dram_tensor("ident", [128, 128], F32R, kind="ExternalInput")
    idb_d = nc.dram_tensor("ident_bf", [128, 128], BF16, kind="ExternalInput")
    mask_d = nc.dram_tensor("mask4", [128, 4, 128], F32, kind="ExternalInput")
    out_d = nc.dram_tensor("out", [TOK, DIM], F32, kind="ExternalOutput")

    qt_d = nc.dram_tensor("qt_i", [H, D, TOK], F32R, kind="Internal")
    kt_d = nc.dram_tensor("kt_i", [H, D, TOK], F32R, kind="Internal")
    # V spill, token-interleaved bf16: [tmod 4, g 32, d 128, tok//4]
    vt_d = nc.dram_tensor("vt_i", [4, H, D, TOK // 4], BF16, kind="Internal")

    with tile.TileContext(nc) as tc:
        with tc.tile_pool(name="consts", bufs=1) as constp:
            id_sb = constp.tile([128, 128], F32R)
            idb_sb = constp.tile([128, 128], BF16)
            mask_sb = constp.tile([128, 4, 128], F32)
            nc.sync.dma_start(out=id_sb[:, :], in_=id_d[:, :])
            nc.sync.dma_start(out=idb_sb[:, :], in_=idb_d[:, :])
            nc.sync.dma_start(out=mask_sb[:, :, :], in_=mask_d[:, :, :])

            # ---------- Phase A: projections ----------
            with tc.tile_pool(name="xT", bufs=1) as xtp:
                xT = xtp.tile([128, KC, TOK], F32R)  # 128 KB/partition
                xt_r = xt_d[:, :].rearrange("(kc p) t -> p kc t", p=128)
                for c4 in range(KC // 4):
                    # alternate the two HW DMA queues (SP / Act) so the
                    # 16MB xT load streams at ~2x one queue's bandwidth
                    eng = nc.sync if c4 % 2 == 0 else nc.scalar
                    eng.dma_start(
                        out=xT[:, c4 * 4 : (c4 + 1) * 4, :],
                        in_=xt_r[:, c4 * 4 : (c4 + 1) * 4, :],
                    )

                with (
                    tc.tile_pool(name="wb", bufs=3) as wbp,
                    tc.tile_pool(name="stA", bufs=3) as stp,
                    tc.tile_pool(name="aps", bufs=3, space="PSUM") as aps,
                ):
                    for w_d, o_d in ((wq_d, qt_d), (wk_d, kt_d), (wv_d, vt_d)):
                        w_r = w_d[:, :].rearrange("(kc c) f -> c kc f", c=128)
                        for F in range(KC):
                            wb = wbp.tile([128, KC, 128], F32R, tag="wb")
                            nc.sync.dma_start(
                                out=wb[:, :, :],
                                in_=w_r[:, :, F * 128 : (F + 1) * 128],
                            )
                            for th in range(TOK // 512):
                                ps = aps.tile([128, 512], F32, tag="aps")
                                for kc in range(KC):
                                    nc.tensor.matmul(
                                        ps[:, :],
                                        wb[:, kc, :],
                                        xT[:, kc, th * 512 : (th + 1) * 512],
                                        start=(kc == 0),
                                        stop=(kc == KC - 1),
                                    )
                                # spills go out on the Act HW DMA queue,
                                # leaving the SP queue for the W stream
                                if o_d is vt_d:
                                    # token-interleave on the (otherwise
                                    # idle) vector engine; spill DMA is then
                                    # contiguous 256B bf16 runs
                                    st2 = stp.tile([128, 4, 128], BF16, tag="stv")
                                    nc.vector.tensor_copy(
                                        st2[:, :, :],
                                        ps[:, :].rearrange(
                                            "d (j tm) -> d tm j", tm=4
                                        ),
                                    )
                                    dst = o_d[
                                        :, F, :, th * 128 : (th + 1) * 128
                                    ].rearrange("tm d j -> d tm j")
                                    nc.scalar.dma_start(out=dst, in_=st2[:, :, :])
                                else:
                                    st = stp.tile([128, 512], F32R, tag="st")
                                    nc.scalar.copy(st[:, :], ps[:, :])
                                    nc.scalar.dma_start(
                                        out=o_d[F, :, th * 512 : (th + 1) * 512],
                                        in_=st[:, :],
                                    )

            # ---------- Phase B + C per 512-token half ----------
            qt_r = qt_d[:, :, :].rearrange("h d t -> d h t")
            kt_r = kt_d[:, :, :].rearrange("h d t -> d h t")
            vt_r = vt_d[:, :, :, :].rearrange("tm g d j -> (tm g) d j")
            wo_r = wo_d[:, :].rearrange("(kc c) f -> c kc f", c=128)

            with tc.tile_pool(name="yt", bufs=1) as ytp:
                for half in range(TOK // HALF):
                    yt = ytp.tile([128, KC, HALF], BF16, tag="yt")
                    h0 = half * HALF

                    with (
                        tc.tile_pool(name="qp", bufs=3) as qpp,
                        tc.tile_pool(name="kp", bufs=3) as kpp,
                        tc.tile_pool(name="vp", bufs=2) as vpp,
                        tc.tile_pool(name="q2p", bufs=3) as q2pp,
                        tc.tile_pool(name="smax", bufs=4) as smp,
                        tc.tile_pool(name="esm", bufs=3) as esp,
                        tc.tile_pool(name="asb", bufs=4) as asp,
                        tc.tile_pool(name="atsb", bufs=3) as atp,
                        tc.tile_pool(name="psS", bufs=2, space="PSUM") as psS,
                        tc.tile_pool(name="psT", bufs=2, space="PSUM") as psT,
                        tc.tile_pool(name="psY", bufs=2, space="PSUM") as psY,
                    ):
                        macro_tiles = {}
                        block_tiles = {}
                        state = {}
                        NSG = GH // 4  # 4-group super-bundles per half (32)

                        def load_macro(m):
                            m0 = h0 + m * MACRO
                            q_sl = qpp.tile([128, H, MACRO], F32R, tag="q")
                            k_sl = kpp.tile([128, H, MACRO], F32R, tag="k")
                            v_sl = vpp.tile([128, D, NG], BF16, tag="v")
                            nc.sync.dma_start(
                                out=q_sl[:, :, :], in_=qt_r[:, :, m0 : m0 + MACRO]
                            )
                            nc.sync.dma_start(
                                out=k_sl[:, :, :], in_=kt_r[:, :, m0 : m0 + MACRO]
                            )
                            nc.sync.dma_start(
                                out=v_sl[:, :, :],
                                in_=vt_r[:, :, m0 // 4 : m0 // 4 + NG],
                            )
                            macro_tiles[m] = (q_sl, k_sl, v_sl)

                        def repack_q(hb):
                            # q2 [d, j, t, h]: contiguous (t,h)=128 per group
                            # so the S-matmul lhsT has a single free dim
                            q_sl, _, _ = macro_tiles[hb * HB // NG]
                            joff = (hb % (NG // HB)) * HB
                            q2 = q2pp.tile([128, HB, 4, H], F32R, tag="q2")
                            q_r = q_sl[:, :, :].rearrange(
                                "d h (j t) -> d t j h", t=4
                            )
                            for tp in range(4):
                                nc.scalar.copy(
                                    q2[:, :, tp, :],
                                    q_r[:, tp, joff : joff + HB, :],
                                )
                            block_tiles[hb] = q2

                        load_macro(0)
                        load_macro(1)
                        load_macro(2)
                        repack_q(0)

                        for i in range(NSG + 4):
                            # macro = 8 SGs; q2 block = 4 SGs
                            if i % 8 == 0 and i // 8 + 3 < HALF // MACRO:
                                load_macro(i // 8 + 3)
                            if i % 4 == 2 and i // 4 + 1 < GH // HB:
                                repack_q(i // 4 + 1)

                            # --- stage S: 4 S-matmuls + batched softmax
                            if i < NSG:
                                _, k_sl, v_sl = macro_tiles[i // 8]
                                q2 = block_tiles[i // 4]
                                s4 = psS.tile([128, 512], F32, tag="s")
                                for g in range(4):
                                    gg = i * 4 + g  # group within half
                                    t0 = (gg % NG) * 4
                                    nc.tensor.matmul(
                                        s4[:, 128 * g : 128 * g + 128],
                                        q2[:, gg % HB, :, :],
                                        k_sl[:, :, t0 : t0 + 4].rearrange(
                                            "d h t -> d t h"
                                        ),
                                        start=True,
                                        stop=True,
                                    )
                                sm4 = esp.tile([128, 512], F32, tag="sm")
                                negmax = smp.tile([128, 1], F32, tag="nm")
                                dn4 = smp.tile([128, 4], F32, tag="dn")
                                rc4 = smp.tile([128, 4], F32, tag="rc")
                                e4 = esp.tile([128, 512], BF16, tag="e")
                                a4 = asp.tile([128, 4, 128], BF16, tag="a")
                                # sm = S + M (mask -1e30 off-block-diagonal).
                                # One shared row-max across the 4 groups is
                                # exact (softmax is shift-invariant per row).
                                nc.vector.tensor_tensor(
                                    sm4[:, :],
                                    s4[:, :],
                                    mask_sb[:, :, :].rearrange(
                                        "p g c -> p (g c)"
                                    ),
                                    mybir.AluOpType.add,
                                )
                                nc.vector.reduce_max(
                                    negmax[:, :],
                                    sm4[:, :],
                                    axis=mybir.AxisListType.X,
                                    negate=True,
                                )
                                nc.scalar.activation(
                                    e4[:, :],
                                    sm4[:, :],
                                    mybir.ActivationFunctionType.Exp,
                                    bias=negmax[:, :],
                                )
                                nc.vector.reduce_sum(
                                    dn4[:, :],
                                    e4[:, :].rearrange("p (g c) -> p g c", g=4),
                                    axis=mybir.AxisListType.X,
                                )
                                nc.vector.reciprocal(rc4[:, :], dn4[:, :])
                                for g in range(4):
                                    nc.vector.tensor_scalar_mul(
                                        a4[:, g, :],
                                        e4[:, 128 * g : 128 * g + 128],
                                        rc4[:, g : g + 1],
                                    )
                                state[i] = a4

                            # --- stage T: 4 transposes for SG i-2
                            g1 = i - 2
                            if 0 <= g1 < NSG:
                                a4 = state.pop(g1)
                                at4_ps = psT.tile([128, 4, 128], BF16, tag="at")
                                for g in range(4):
                                    nc.tensor.transpose(
                                        at4_ps[:, g, :], a4[:, g, :], idb_sb[:, :]
                                    )
                                at4 = atp.tile([128, 4, 128], BF16, tag="ats")
                                nc.scalar.copy(at4[:, :, :], at4_ps[:, :, :])
                                state[("at", g1)] = at4

                            # --- stage Y: 4 y-matmuls + yt copy for SG i-3
                            g2 = i - 3
                            if 0 <= g2 < NSG:
                                at4 = state.pop(("at", g2))
                                _, _, v_sl = macro_tiles[g2 // 8]
                                y4_ps = psY.tile([128, 4, 128], F32, tag="y")
                                for g in range(4):
                                    gg = g2 * 4 + g
                                    nc.tensor.matmul(
                                        y4_ps[:, g, :],
                                        v_sl[:, :, gg % NG],
                                        at4[:, g, :],
                                        start=True,
                                        stop=True,
                                    )
                                dst = yt[
                                    :, :, g2 * 16 : g2 * 16 + 16
                                ].rearrange("d h (g t) -> d h g t", g=4)
                                src = y4_ps[:, :, :].rearrange(
                                    "p g (t h) -> p h g t", t=4
                                )
                                nc.scalar.copy(dst, src)

                    with (
                        tc.tile_pool(name="wob", bufs=2) as wop,
                        tc.tile_pool(name="stC", bufs=3) as stc,
                        tc.tile_pool(name="cps", bufs=3, space="PSUM") as cps,
                    ):
                        for fo in range(DIM // 512):
                            wob = wop.tile([128, KC, 512], BF16, tag="wob")
                            nc.sync.dma_start(
                                out=wob[:, :, :],
                                in_=wo_r[:, :, fo * 512 : (fo + 1) * 512],
                            )
                            for tt in range(HALF // 128):
                                ps = cps.tile([128, 512], F32, tag="cps")
                                for kc in range(KC):
                                    nc.tensor.matmul(
                                        ps[:, :],
                                        yt[:, kc, tt * 128 : (tt + 1) * 128],
                                        wob[:, kc, :],
                                        start=(kc == 0),
                                        stop=(kc == KC - 1),
                                    )
                                st = stc.tile([128, 512], F32, tag="stc")
                                nc.scalar.copy(st[:, :], ps[:, :])
                                nc.scalar.dma_start(
                                    out=out_d[
                                        h0 + tt * 128 : h0 + (tt + 1) * 128,
                                        fo * 512 : (fo + 1) * 512,
                                    ],
                                    in_=st[:, :],
                                )

    nc.compile()
    return nc


def _get_nc():
    if "nc" not in _NC_CACHE:
        _NC_CACHE["nc"] = _build_nc()
    return _NC_CACHE["nc"]


def kernel(x, W_q, W_k, W_v, W_o):
    x = np.ascontiguousarray(x, dtype=np.float32)
    W_q = np.ascontiguousarray(W_q, dtype=np.float32)
    W_k = np.ascontiguousarray(W_k, dtype=np.float32)
    W_v = np.ascontiguousarray(W_v, dtype=np.float32)
    W_o = np.ascontiguousarray(W_o, dtype=np.float32)

    wo_bf16 = W_o.astype(ml_dtypes.bfloat16)
    ident = np.eye(128, dtype=np.float32)
    ident_bf = np.eye(128, dtype=np.float32).astype(ml_dtypes.bfloat16)
    # -1e30 off the 32x32 block diagonal (block = token within the group),
    # tiled x4 for the super-group [128, 512] scores tile
    blk = np.arange(128) // 32
    mask = np.where(blk[:, None] == blk[None, :], 0.0, -1.0e30).astype(np.float32)
    mask4 = np.ascontiguousarray(
        np.broadcast_to(mask[:, None, :], (128, 4, 128))
    )
    xt_full = np.ascontiguousarray(x.T)  # [DIM, N]

    nc = _get_nc()
    in_maps = []
    for c in range(N_CORES):
        in_maps.append(
            {
                "xt": np.ascontiguousarray(xt_full[:, c * TOK : (c + 1) * TOK]),
                "wq": W_q,
                "wk": W_k,
                "wv": W_v,
                "wo_bf16": wo_bf16,
                "ident": ident,
                "ident_bf": ident_bf,
                "mask4": mask4,
            }
        )
    trace = bool(int(os.environ.get("KERNEL_TRACE", "0")))
    res = run_bass_kernel_spmd(
        nc, in_maps, core_ids=list(range(N_CORES)), trace=trace
    )
    if trace:
        kernel.last_exec_time_ns = res.exec_time_ns
        kernel.last_results = res
    out = np.concatenate([r["out"] for r in res.results], axis=0)
    return np.ascontiguousarray(out, dtype=np.float32)


# revision 31
# speedup vs baseline: 1.1950x; 1.1950x over previous
"""TRN2 Bass kernel for nn_MultiHeadAttention_79714593014244.

Reference math (per token n, NOT sequence attention):
    Q = x @ W_q, K = x @ W_k, V = x @ W_v          (x: [N, 4096])
    S[n] = Q[n] @ K[n].T        over heads          ([32, 32] per token)
    A[n] = softmax(S[n], axis=-1)
    y[n] = A[n] @ V[n]
    out = y.reshape(N, 4096) @ W_o
Sharding: pure data-parallel over tokens across 8 cores.

Per-core plan (projections fp32r, W_o pass bf16):
  Phase A: xT [128, 32 kc, 1024 tok] DMA'd directly (x host-transposed).
           For W in (W_q, W_k, W_v): stream W column-chunks, compute
           [feat, tok] projections (PSUM-accumulated over 32 K-chunks),
           spill to DRAM. Q/K spill as [head, d, tok]; V spills
           token-interleaved as [4*32 (tmod, g), d, tok/4] so that a
           4-token V block later loads as 128 partitions directly.
  Phase B (per 512-token half): attention, 4 tokens per PE instruction.
           For each group of 4 tokens:
             - ONE matmul S[(t,h) 128, (t',g) 128] (contract d=128) via
               strided APs picking 4 tokens from the [d, h, tok] slices.
               Only diagonal 32x32 blocks are wanted; off-diag is
               cross-token garbage.
             - DVE tensor_tensor_reduce: sm = -(S + M) with M the
               -1e30 off-block-diagonal mask, accum min -> negmax
               (= -(per-row max over the valid block)).
             - ONE scalar Exp [128,128]: e = exp(-sm + negmax), i.e.
               exp(S+M - max); off-diag -> exp(-1e30)=0; accum -> denom.
             - DVE reciprocal; Pool tensor_scalar_mul: a = e * recip
               (fp32r, block-diagonal).
             - ONE PE transpose a -> aT [(t',g), (t,h)].
             - ONE matmul yT[d 128, (t,h) 128] = V4.T @ aT (contract
               (t',g)=128) with V4 from the interleaved spill.
             - Pool copy into yt [128 d, 32 h, 512 tok] (bf16).
           PE work is software-pipelined: transpose/y-matmul for group
           g are emitted LAG groups behind the S-matmul so the PE queue
           never stalls on the softmax round-trip.
  Phase C (per half): stream bf16 W_o column-chunks, out = y @ W_o.
"""

import os

import ml_dtypes
import numpy as np

import concourse.bass as bass
import concourse.tile as tile
from concourse import bacc, mybir
from concourse.bass_utils import run_bass_kernel_spmd

N_CORES = 8
N_TOKENS = 8192
DIM = 4096
H = 32  # heads
D = 128  # head dim
KC = DIM // 128  # contraction chunks (32)
TOK = N_TOKENS // N_CORES  # tokens per core (1024)
HALF = 512  # tokens per B+C fusion block
MACRO = 128  # tokens per attention slice load
NG = MACRO // 4  # 4-token groups per macro (32)
GH = HALF // 4  # groups per half (128)
HB = 16  # groups per Q-repack block
LAG = 4  # PE pipeline lag (groups) between S-matmul and transpose
F32R = mybir.dt.float32r
F32 = mybir.dt.float32
BF16 = mybir.dt.bfloat16

_NC_CACHE = {}


def _build_nc():
    nc = bacc.Bacc(None, target_bir_lowering=False)

    xt_d = nc.dram_tensor("xt", [DIM, TOK], F32R, kind="ExternalInput")
    wq_d = nc.dram_tensor("wq", [DIM, DIM], F32R, kind="ExternalInput")
    wk_d = nc.dram_tensor("wk", [DIM, DIM], F32R, kind="ExternalInput")
    wv_d = nc.dram_tensor("wv", [DIM, DIM], F32R, kind="ExternalInput")
    wo_d = nc.dram_tensor("wo_bf16", [DIM, DIM], BF16, kind="ExternalInput")
    id_d = nc.dram_tensor("ident", [128, 128], F32R, kind="ExternalInput")
    idb_d = nc.dram_tensor("ident_bf", [128, 128], BF16, kind="ExternalInput")
    mask_d = nc.dram_tensor("mask4", [128, 4, 128], F32, kind="ExternalInput")
    out_d = nc.dram_tensor("out", [TOK, DIM], F32, kind="ExternalOutput")

    qt_d = nc.dram_tensor("qt_i", [H, D, TOK], F32R, kind="Internal")
    kt_d = nc.dram_tensor("kt_i", [H, D, TOK], F32R, kind="Internal")
    # V spill, token-interleaved bf16: [tmod 4, g 32, d 128, tok//4]
    vt_d = nc.dram_tensor("vt_i", [4, H, D, TOK // 4], BF16, kind="Internal")

    with tile.TileContext(nc) as tc:
        with tc.tile_pool(name="consts", bufs=1) as constp:
            id_sb = constp.tile([128, 128], F32R)
            idb_sb = constp.tile([128, 128], BF16)
            mask_sb = constp.tile([128, 4, 128], F32)
            nc.sync.dma_start(out=id_sb[:, :], in_=id_d[:, :])
            nc.sync.dma_start(out=idb_sb[:, :], in_=idb_d[:, :])
            nc.sync.dma_start(out=mask_sb[:, :, :], in_=mask_d[:, :, :])

            # ---------- Phase A: projections ----------
            with tc.tile_pool(name="xT", bufs=1) as xtp:
                xT = xtp.tile([128, KC, TOK], F32R)  # 128 KB/partition
                xt_r = xt_d[:, :].rearrange("(kc p) t -> p kc t", p=128)

                with (
                    tc.tile_pool(name="wb", bufs=3) as wbp,
                    tc.tile_pool(name="stA", bufs=3) as stp,
                    tc.tile_pool(name="aps", bufs=3, space="PSUM") as aps,
                ):
                    # prefetch the first two W_q chunks ahead of the xT
                    # stream (one per HW DMA queue) so the PE pipeline
                    # warms as soon as xT lands
                    wq_r = wq_d[:, :].rearrange("(kc c) f -> c kc f", c=128)
                    wb_pre = []
                    for F, eng in ((0, nc.sync), (1, nc.scalar)):
                        wbp_t = wbp.tile([128, KC, 128], F32R, tag="wb")
                        eng.dma_start(
                            out=wbp_t[:, :, :],
                            in_=wq_r[:, :, F * 128 : (F + 1) * 128],
                        )
                        wb_pre.append(wbp_t)
                    for c4 in range(KC // 4):
                        # alternate the two HW DMA queues (SP / Act) so the
                        # 16MB xT load streams at ~2x one queue's bandwidth
                        eng = nc.sync if c4 % 2 == 0 else nc.scalar
                        eng.dma_start(
                            out=xT[:, c4 * 4 : (c4 + 1) * 4, :],
                            in_=xt_r[:, c4 * 4 : (c4 + 1) * 4, :],
                        )

                    for w_d, o_d in ((wq_d, qt_d), (wk_d, kt_d), (wv_d, vt_d)):
                        w_r = w_d[:, :].rearrange("(kc c) f -> c kc f", c=128)
                        for F in range(KC):
                            if w_d is wq_d and F < 2:
                                wb = wb_pre[F]
                            else:
                                wb = wbp.tile([128, KC, 128], F32R, tag="wb")
                                nc.sync.dma_start(
                                    out=wb[:, :, :],
                                    in_=w_r[:, :, F * 128 : (F + 1) * 128],
                                )
                            for th in range(TOK // 512):
                                ps = aps.tile([128, 512], F32, tag="aps")
                                for kc in range(KC):
                                    nc.tensor.matmul(
                                        ps[:, :],
                                        wb[:, kc, :],
                                        xT[:, kc, th * 512 : (th + 1) * 512],
                                        start=(kc == 0),
                                        stop=(kc == KC - 1),
                                    )
                                # spills go out on the Act HW DMA queue,
                                # leaving the SP queue for the W stream
                                if o_d is vt_d:
                                    # token-interleave on the (otherwise
                                    # idle) vector engine; spill DMA is then
                                    # contiguous 256B bf16 runs
                                    st2 = stp.tile([128, 4, 128], BF16, tag="stv")
                                    nc.vector.tensor_copy(
                                        st2[:, :, :],
                                        ps[:, :].rearrange(
                                            "d (j tm) -> d tm j", tm=4
                                        ),
                                    )
                                    dst = o_d[
                                        :, F, :, th * 128 : (th + 1) * 128
                                    ].rearrange("tm d j -> d tm j")
                                    nc.scalar.dma_start(out=dst, in_=st2[:, :, :])
                                else:
                                    st = stp.tile([128, 512], F32R, tag="st")
                                    nc.scalar.copy(st[:, :], ps[:, :])
                                    nc.scalar.dma_start(
                                        out=o_d[F, :, th * 512 : (th + 1) * 512],
                                        in_=st[:, :],
                                    )

            # ---------- Phase B + C per 512-token half ----------
            qt_r = qt_d[:, :, :].rearrange("h d t -> d h t")
            kt_r = kt_d[:, :, :].rearrange("h d t -> d h t")
            vt_r = vt_d[:, :, :, :].rearrange("tm g d j -> (tm g) d j")
            wo_r = wo_d[:, :].rearrange("(kc c) f -> c kc f", c=128)

            with tc.tile_pool(name="yt", bufs=1) as ytp:
                for half in range(TOK // HALF):
                    yt = ytp.tile([128, KC, HALF], BF16, tag="yt")
                    h0 = half * HALF

                    with (
                        tc.tile_pool(name="qp", bufs=3) as qpp,
                        tc.tile_pool(name="kp", bufs=3) as kpp,
                        tc.tile_pool(name="vp", bufs=2) as vpp,
                        tc.tile_pool(name="q2p", bufs=3) as q2pp,
                        tc.tile_pool(name="smax", bufs=4) as smp,
                        tc.tile_pool(name="esm", bufs=3) as esp,
                        tc.tile_pool(name="asb", bufs=4) as asp,
                        tc.tile_pool(name="atsb", bufs=3) as atp,
                        tc.tile_pool(name="psS", bufs=2, space="PSUM") as psS,
                        tc.tile_pool(name="psT", bufs=2, space="PSUM") as psT,
                        tc.tile_pool(name="psY", bufs=2, space="PSUM") as psY,
                    ):
                        macro_tiles = {}
                        block_tiles = {}
                        state = {}
                        NSG = GH // 4  # 4-group super-bundles per half (32)

                        def load_macro(m):
                            m0 = h0 + m * MACRO
                            q_sl = qpp.tile([128, H, MACRO], F32R, tag="q")
                            k_sl = kpp.tile([128, H, MACRO], F32R, tag="k")
                            v_sl = vpp.tile([128, D, NG], BF16, tag="v")
                            nc.sync.dma_start(
                                out=q_sl[:, :, :], in_=qt_r[:, :, m0 : m0 + MACRO]
                            )
                            nc.sync.dma_start(
                                out=k_sl[:, :, :], in_=kt_r[:, :, m0 : m0 + MACRO]
                            )
                            nc.sync.dma_start(
                                out=v_sl[:, :, :],
                                in_=vt_r[:, :, m0 // 4 : m0 // 4 + NG],
                            )
                            macro_tiles[m] = (q_sl, k_sl, v_sl)

                        def repack_q(hb):
                            # q2 [d, j, t, h]: contiguous (t,h)=128 per group
                            # so the S-matmul lhsT has a single free dim
                            q_sl, _, _ = macro_tiles[hb * HB // NG]
                            joff = (hb % (NG // HB)) * HB
                            q2 = q2pp.tile([128, HB, 4, H], F32R, tag="q2")
                            q_r = q_sl[:, :, :].rearrange(
                                "d h (j t) -> d t j h", t=4
                            )
                            for tp in range(4):
                                nc.scalar.copy(
                                    q2[:, :, tp, :],
                                    q_r[:, tp, joff : joff + HB, :],
                                )
                            block_tiles[hb] = q2

                        load_macro(0)
                        load_macro(1)
                        load_macro(2)
                        repack_q(0)

                        for i in range(NSG + 4):
                            # macro = 8 SGs; q2 block = 4 SGs
                            if i % 8 == 0 and i // 8 + 3 < HALF // MACRO:
                                load_macro(i // 8 + 3)
                            if i % 4 == 2 and i // 4 + 1 < GH // HB:
                                repack_q(i // 4 + 1)

                            # --- stage S: 4 S-matmuls + batched softmax
                            if i < NSG:
                                _, k_sl, v_sl = macro_tiles[i // 8]
                                q2 = block_tiles[i // 4]
                                s4 = psS.tile([128, 512], F32, tag="s")
                                for g in range(4):
                                    gg = i * 4 + g  # group within half
                                    t0 = (gg % NG) * 4
                                    nc.tensor.matmul(
                                        s4[:, 128 * g : 128 * g + 128],
                                        q2[:, gg % HB, :, :],
                                        k_sl[:, :, t0 : t0 + 4].rearrange(
                                            "d h t -> d t h"
                                        ),
                                        start=True,
                                        stop=True,
                                    )
                                sm4 = esp.tile([128, 512], F32, tag="sm")
                                negmax = smp.tile([128, 1], F32, tag="nm")
                                dn4 = smp.tile([128, 4], F32, tag="dn")
                                rc4 = smp.tile([128, 4], F32, tag="rc")
                                e4 = esp.tile([128, 512], BF16, tag="e")
                                a4 = asp.tile([128, 4, 128], BF16, tag="a")
                                # sm = S + M (mask -1e30 off-block-diagonal).
                                # One shared row-max across the 4 groups is
                                # exact (softmax is shift-invariant per row).
                                nc.vector.tensor_tensor(
                                    sm4[:, :],
                                    s4[:, :],
                                    mask_sb[:, :, :].rearrange(
                                        "p g c -> p (g c)"
                                    ),
                                    mybir.AluOpType.add,
                                )
                                nc.vector.reduce_max(
                                    negmax[:, :],
                                    sm4[:, :],
                                    axis=mybir.AxisListType.X,
                                    negate=True,
                                )
                                nc.scalar.activation(
                                    e4[:, :],
                                    sm4[:, :],
                                    mybir.ActivationFunctionType.Exp,
                                    bias=negmax[:, :],
                                )
                                nc.vector.reduce_sum(
                                    dn4[:, :],
                                    e4[:, :].rearrange("p (g c) -> p g c", g=4),
                                    axis=mybir.AxisListType.X,
                                )
                                nc.vector.reciprocal(rc4[:, :], dn4[:, :])
                                for g in range(4):
                                    nc.vector.tensor_scalar_mul(
                                        a4[:, g, :],
                                        e4[:, 128 * g : 128 * g + 128],
                                        rc4[:, g : g + 1],
                                    )
                                state[i] = a4

                            # --- stage T: 4 transposes for SG i-2
                            g1 = i - 2
                            if 0 <= g1 < NSG:
                                a4 = state.pop(g1)
                                at4_ps = psT.tile([128, 4, 128], BF16, tag="at")
                                for g in range(4):
                                    nc.tensor.transpose(
                                        at4_ps[:, g, :], a4[:, g, :], idb_sb[:, :]
                                    )
                                at4 = atp.tile([128, 4, 128], BF16, tag="ats")
                                nc.scalar.copy(at4[:, :, :], at4_ps[:, :, :])
                                state[("at", g1)] = at4

                            # --- stage Y: 4 y-matmuls + yt copy for SG i-3
                            g2 = i - 3
                            if 0 <= g2 < NSG:
                                at4 = state.pop(("at", g2))
                                _, _, v_sl = macro_tiles[g2 // 8]
                                y4_ps = psY.tile([128, 4, 128], F32, tag="y")
                                for g in range(4):
                                    gg = g2 * 4 + g
                                    nc.tensor.matmul(
                                        y4_ps[:, g, :],
                                        v_sl[:, :, gg % NG],
                                        at4[:, g, :],
                                        start=True,
                                        stop=True,
                                    )
                                for g in range(4):
                                    tok0 = g2 * 16 + g * 4
                                    nc.scalar.copy(
                                        yt[:, :, tok0 : tok0 + 4],
                                        y4_ps[:, g, :].rearrange(
                                            "p (t h) -> p h t", t=4
                                        ),
                                    )

                    with (
                        tc.tile_pool(name="wob", bufs=2) as wop,
                        tc.tile_pool(name="stC", bufs=3) as stc,
                        tc.tile_pool(name="cps", bufs=3, space="PSUM") as cps,
                    ):
                        for fo in range(DIM // 512):
                            wob = wop.tile([128, KC, 512], BF16, tag="wob")
                            nc.sync.dma_start(
                                out=wob[:, :, :],
                                in_=wo_r[:, :, fo * 512 : (fo + 1) * 512],
                            )
                            for tt in range(HALF // 128):
                                ps = cps.tile([128, 512], F32, tag="cps")
                                for kc in range(KC):
                                    nc.tensor.matmul(
                                        ps[:, :],
                                        yt[:, kc, tt * 128 : (tt + 1) * 128],
                                        wob[:, kc, :],
                                        start=(kc == 0),
                                        stop=(kc == KC - 1),
                                    )
                                st = stc.tile([128, 512], F32, tag="stc")
                                nc.scalar.copy(st[:, :], ps[:, :])
                                nc.scalar.dma_start(
                                    out=out_d[
                                        h0 + tt * 128 : h0 + (tt + 1) * 128,
                                        fo * 512 : (fo + 1) * 512,
                                    ],
                                    in_=st[:, :],
                                )

    nc.compile()
    return nc


def _get_nc():
    if "nc" not in _NC_CACHE:
        _NC_CACHE["nc"] = _build_nc()
    return _NC_CACHE["nc"]


def kernel(x, W_q, W_k, W_v, W_o):
    x = np.ascontiguousarray(x, dtype=np.float32)
    W_q = np.ascontiguousarray(W_q, dtype=np.float32)
    W_k = np.ascontiguousarray(W_k, dtype=np.float32)
    W_v = np.ascontiguousarray(W_v, dtype=np.float32)
    W_o = np.ascontiguousarray(W_o, dtype=np.float32)

    wo_bf16 = W_o.astype(ml_dtypes.bfloat16)
    ident = np.eye(128, dtype=np.float32)
    ident_bf = np.eye(128, dtype=np.float32).astype(ml_dtypes.bfloat16)
    # -1e30 off the 32x32 block diagonal (block = token within the group),
    # tiled x4 for the super-group [128, 512] scores tile
    blk = np.arange(128) // 32
    mask = np.where(blk[:, None] == blk[None, :], 0.0, -1.0e30).astype(np.float32)
    mask4 = np.ascontiguousarray(
        np.broadcast_to(mask[:, None, :], (128, 4, 128))
    )
    xt_full = np.ascontiguousarray(x.T)  # [DIM, N]

    nc = _get_nc()
    in_maps = []
    for c in range(N_CORES):
        in_maps.append(
            {
                "xt": np.ascontiguousarray(xt_full[:, c * TOK : (c + 1) * TOK]),
                "wq": W_q,
                "wk": W_k,
                "wv": W_v,
                "wo_bf16": wo_bf16,
                "ident": ident,
                "ident_bf": ident_bf,
                "mask4": mask4,
            }
        )
    trace = bool(int(os.environ.get("KERNEL_TRACE", "0")))
    res = run_bass_kernel_spmd(
        nc, in_maps, core_ids=list(range(N_CORES)), trace=trace
    )
    if trace:
        kernel.last_exec_time_ns = res.exec_time_ns
        kernel.last_results = res
    out = np.concatenate([r["out"] for r in res.results], axis=0)
    return np.ascontiguousarray(out, dtype=np.float32)
